# revision 5
# baseline (speedup 1.0000x reference)
"""Trainium2 Bass kernel for a 3-layer GRU (B=128, T=512, IN=128, H=1024, PRED=96).

Strategy: data-parallel over batch across 8 NeuronCores (B_core=16).
Per core, per layer:
  phase A (inproj): xi = W_ih @ h_prev_seq as batched N=512 GEMMs (bf16),
      biases (b_ih, + b_hh for r/z gates) folded into the PSUM->SBUF
      evacuation on the scalar engine; xi round-trips through DRAM.
  phase B (recurrence): 512 sequential steps; per step 192 self-loading
      bf16 matmuls (K=128, M=128, N=16) accumulate W_hh @ h_t into one
      PSUM bank laid out gate-major [128, 24 chunks x 16 batch]; gate
      math on DVE/ACT in the same layout; hidden state kept fp32 with a
      bf16 shadow for the tensor engine.
Final FC done on-chip; host only transposes [96,16] -> [16,96] per core.
"""

import numpy as np
from contextlib import ExitStack

import concourse.bass as bass
import concourse.bacc as bacc
import concourse.mybir as mybir
import concourse.tile as tile
from concourse.bass_utils import run_bass_kernel_spmd

try:
    from ml_dtypes import bfloat16 as np_bf16
except ImportError:  # pragma: no cover
    import jax.numpy as jnp

    np_bf16 = jnp.bfloat16

B, T, IN, H, NLAYERS, PRED = 128, 512, 128, 1024, 3, 96
NCORES = 8
BC = B // NCORES  # 16
G = 3 * H  # 3072
NK = H // 128  # 8
NM = G // 128  # 24
BLK = 8  # recurrence steps per For_i iteration
F32, BF16 = mybir.dt.float32, mybir.dt.bfloat16
AF = mybir.ActivationFunctionType


def build(T_=T):
    nt = T_ * BC
    nblk = T_ // BLK
    ntile = nt // 512 if nt >= 512 else 0
    nc = bacc.Bacc("TRN2", target_bir_lowering=False, debug=False,
                   num_devices=NCORES)

    xT = nc.dram_tensor("xT", [128, nt], BF16, kind="ExternalInput")
    wihs = [
        nc.dram_tensor("wih0", [128, 1, G], BF16, kind="ExternalInput"),
        nc.dram_tensor("wih1", [128, NK, G], BF16, kind="ExternalInput"),
        nc.dram_tensor("wih2", [128, NK, G], BF16, kind="ExternalInput"),
    ]
    whhs = [nc.dram_tensor(f"whh{l}", [128, NK, G], BF16, kind="ExternalInput")
            for l in range(NLAYERS)]
    bizs = [nc.dram_tensor(f"biz{l}", [128, NM], F32, kind="ExternalInput")
            for l in range(NLAYERS)]
    bhns = [nc.dram_tensor(f"bhn{l}", [128, NK * BC], F32, kind="ExternalInput")
            for l in range(NLAYERS)]
    fcw = nc.dram_tensor("fcw", [128, NK, PRED], BF16, kind="ExternalInput")
    fcb = nc.dram_tensor("fcb", [PRED, 1], F32, kind="ExternalInput")
    out = nc.dram_tensor("out", [PRED, BC], F32, kind="ExternalOutput")

    with tile.TileContext(nc) as tc, ExitStack() as ctx:
        wpool = ctx.enter_context(tc.tile_pool(name="w", bufs=1))
        cpool = ctx.enter_context(tc.tile_pool(name="const", bufs=1))
        xpool = ctx.enter_context(tc.tile_pool(name="xT", bufs=1))
        rpool = ctx.enter_context(tc.tile_pool(name="rhs", bufs=2))
        epool = ctx.enter_context(tc.tile_pool(name="ev", bufs=3))
        xipool = ctx.enter_context(tc.tile_pool(name="xib", bufs=2))
        wnpool = ctx.enter_context(tc.tile_pool(name="win", bufs=2))
        tpool = ctx.enter_context(tc.tile_pool(name="tmp", bufs=2))
        spool = ctx.enter_context(tc.tile_pool(name="state", bufs=1))
        pspool = ctx.enter_context(tc.tile_pool(name="ps", bufs=3, space="PSUM"))
        fcpool = ctx.enter_context(tc.tile_pool(name="psfc", bufs=1, space="PSUM"))
        pgpool = ctx.enter_context(tc.tile_pool(name="pg", bufs=2, space="PSUM"))
        dpool = ctx.enter_context(tc.tile_pool(name="dram", bufs=1, space="DRAM"))
        hqpool = ctx.enter_context(tc.tile_pool(name="hseq", bufs=2, space="DRAM"))

        # persistent state
        h32 = spool.tile([128, NK, BC], F32, tag="h32")
        h16 = spool.tile([128, NK, BC], BF16, tag="h16")

        xT_sb = xpool.tile([128, nt], BF16, tag="xT")
        nc.sync.dma_start(xT_sb[:], xT[:])
        fcw_sb = cpool.tile([128, NK, PRED], BF16, tag="fcw")
        nc.sync.dma_start(fcw_sb[:], fcw[:])
        fcb_sb = cpool.tile([PRED, 1], F32, tag="fcb")
        nc.sync.dma_start(fcb_sb[:], fcb[:])

        hseq_prev = None
        for l in range(NLAYERS):
            nkl = 1 if l == 0 else NK
            wih_sb = wpool.tile([128, NK, G], BF16, tag="wih")
            nc.sync.dma_start(wih_sb[:, 0:nkl, :], wihs[l][:])
            whh_sb = wpool.tile([128, NK, G], BF16, tag="whh")
            nc.sync.dma_start(whh_sb[:], whhs[l][:])
            biz_sb = cpool.tile([128, NM], F32, tag="biz")
            nc.sync.dma_start(biz_sb[:], bizs[l][:])
            bhn_sb = cpool.tile([128, NK * BC], F32, tag="bhn")
            nc.sync.dma_start(bhn_sb[:], bhns[l][:])

            xi = dpool.tile([128, NM, nt], F32, tag="xi")

            # ---------------- phase A: input projection ----------------
            def inproj_body(n):
                noff = n * 512
                if l == 0:
                    rhs_of = [xT_sb[:, bass.ds(noff, 512)]]
                else:
                    rhsb = rpool.tile([128, NK, 512], BF16, tag="rhs")
                    nc.sync.dma_start(rhsb[:], hseq_prev[:, :, bass.ds(noff, 512)])
                    rhs_of = [rhsb[:, k, :] for k in range(NK)]
                for m in range(NM):
                    ps = pspool.tile([128, 512], F32, tag="ps")
                    for k in range(nkl):
                        nc.tensor.matmul(ps[:], wih_sb[:, k, m * 128:(m + 1) * 128],
                                         rhs_of[k], start=(k == 0), stop=(k == nkl - 1))
                    ev = epool.tile([128, 512], F32, tag="ev")
                    nc.scalar.activation(ev[:], ps[:], AF.Identity, bias=biz_sb[:, m:m + 1])
                    nc.sync.dma_start(xi[:, m, bass.ds(noff, 512)], ev[:])

            if ntile:
                with tc.For_i(0, ntile) as n:
                    inproj_body(n)
            else:  # tiny-T debug path
                for m in range(NM):
                    ps = pspool.tile([128, nt], F32, tag="ps")
                    if l == 0:
                        for k in range(1):
                            nc.tensor.matmul(ps[:], wih_sb[:, k, m * 128:(m + 1) * 128],
                                             xT_sb[:], start=True, stop=True)
                    else:
                        rhsb = rpool.tile([128, NK, nt], BF16, tag="rhs")
                        nc.sync.dma_start(rhsb[:], hseq_prev[:])
                        for k in range(NK):
                            nc.tensor.matmul(ps[:], wih_sb[:, k, m * 128:(m + 1) * 128],
                                             rhsb[:, k, :], start=(k == 0), stop=(k == NK - 1))
                    ev = epool.tile([128, nt], F32, tag="ev")
                    nc.scalar.activation(ev[:], ps[:], AF.Identity, bias=biz_sb[:, m:m + 1])
                    nc.sync.dma_start(xi[:, m, :], ev[:])

            # ---------------- phase B: recurrence ----------------
            nc.vector.memset(h32[:], 0.0)
            nc.vector.memset(h16[:], 0.0)
            last = l == NLAYERS - 1
            if not last:
                hseq = hqpool.tile([128, NK, nt], BF16, tag="hseq")

            with tc.For_i(0, nblk) as blk:
                coff = blk * (BLK * BC)
                xib = xipool.tile([128, NM, BLK * BC], F32, tag="xib")
                nc.sync.dma_start(xib[:], xi[:, :, bass.ds(coff, BLK * BC)])
                if not last:
                    winb = wnpool.tile([128, NK, BLK * BC], BF16, tag="win")
                for dt in range(BLK):
                    pg = pgpool.tile([128, NM, BC], F32, tag="pg")
                    for m in range(NM):
                        for k in range(NK):
                            nc.tensor.matmul(pg[:, m, :],
                                             whh_sb[:, k, m * 128:(m + 1) * 128],
                                             h16[:, k, :],
                                             start=(k == 0), stop=(k == NK - 1))
                    xs = xib[:, :, dt * BC:(dt + 1) * BC]
                    rzp = tpool.tile([128, 256], F32, tag="rzp")
                    nc.vector.tensor_add(rzp[:], pg[:, 0:16, :], xs[:, 0:16, :])
                    rz = tpool.tile([128, 256], F32, tag="rz")
                    nc.scalar.activation(rz[:], rzp[:], AF.Sigmoid)
                    hnb = tpool.tile([128, 128], F32, tag="hnb")
                    nc.vector.tensor_add(hnb[:], pg[:, 16:24, :], bhn_sb[:])
                    t1 = tpool.tile([128, 128], F32, tag="t1")
                    nc.vector.tensor_mul(t1[:], rz[:, 0:128], hnb[:])
                    t2 = tpool.tile([128, 128], F32, tag="t2")
                    nc.vector.tensor_add(t2[:], t1[:], xs[:, 16:24, :])
                    nt_ = tpool.tile([128, 128], F32, tag="nt")
                    nc.scalar.activation(nt_[:], t2[:], AF.Tanh)
                    d = tpool.tile([128, 128], F32, tag="d")
                    nc.vector.tensor_sub(d[:], h32[:], nt_[:])
                    t4 = tpool.tile([128, 128], F32, tag="t4")
                    nc.vector.tensor_mul(t4[:], d[:], rz[:, 128:256])
                    nc.vector.tensor_add(h16[:], t4[:], nt_[:])
                    nc.vector.tensor_add(h32[:], t4[:], nt_[:])
                    if not last:
                        nc.vector.tensor_copy(winb[:, :, dt * BC:(dt + 1) * BC], h16[:])
                if not last:
                    nc.sync.dma_start(hseq[:, :, bass.ds(coff, BLK * BC)], winb[:])
            if not last:
                hseq_prev = hseq

        # ---------------- final FC ----------------
        psfc = fcpool.tile([PRED, BC], F32, tag="psfc")
        for k in range(NK):
            nc.tensor.matmul(psfc[:], fcw_sb[:, k, :], h16[:, k, :],
                             start=(k == 0), stop=(k == NK - 1))
        ofc = epool.tile([PRED, BC], F32, tag="ofc")
        nc.scalar.activation(ofc[:], psfc[:], AF.Identity, bias=fcb_sb[:])
        nc.sync.dma_start(out[:], ofc[:])

    nc.compile()
    return nc


def prep_inputs(inputs, T_=T):
    """Host-side layout preprocessing. Returns (in_maps, shared)."""
    x = np.asarray(inputs["x"], np.float32)

    def chunkT(w):  # [G_out, K*128] -> [128, K, G_out] (lhsT layout)
        w = np.asarray(w, np.float32)
        gout, kin = w.shape
        return np.ascontiguousarray(
            w.T.reshape(kin // 128, 128, gout).transpose(1, 0, 2)
        ).astype(np_bf16)

    shared = {}
    for l in range(NLAYERS):
        wih = np.asarray(inputs[f"w_ih_{l}"], np.float32)
        shared[f"wih{l}"] = chunkT(wih)
        shared[f"whh{l}"] = chunkT(inputs[f"w_hh_{l}"])
        b_ih = np.asarray(inputs[f"b_ih_{l}"], np.float32)
        b_hh = np.asarray(inputs[f"b_hh_{l}"], np.float32)
        comb = b_ih.copy()
        comb[:2 * H] += b_hh[:2 * H]
        shared[f"biz{l}"] = np.ascontiguousarray(comb.reshape(NM, 128).T)
        bhn = b_hh[2 * H:].reshape(NK, 128).T  # [128, NK]
        shared[f"bhn{l}"] = np.ascontiguousarray(np.repeat(bhn, BC, axis=1))
    shared["fcw"] = chunkT(inputs["fc_w"])
    shared["fcb"] = np.asarray(inputs["fc_b"], np.float32).reshape(PRED, 1)

    in_maps = []
    for c in range(NCORES):
        xc = x[c * BC:(c + 1) * BC, :T_, :]  # [BC, T, IN]
        xTc = np.ascontiguousarray(
            xc.transpose(2, 1, 0).reshape(IN, T_ * BC)
        ).astype(np_bf16)
        m = dict(shared)
        m["xT"] = xTc
        in_maps.append(m)
    return in_maps


_NC_CACHE = {}


def kernel(**inputs):
    if "nc" not in _NC_CACHE:
        _NC_CACHE["nc"] = build()
    nc = _NC_CACHE["nc"]
    in_maps = prep_inputs(inputs)
    res = run_bass_kernel_spmd(nc, in_maps, list(range(NCORES)))
    outs = []
    for c in range(NCORES):
        o = np.asarray(res.results[c]["out"], np.float32)  # [PRED, BC]
        outs.append(o.T)  # [BC, PRED]
    return np.concatenate(outs, axis=0)  # [B, PRED]


if __name__ == "__main__":
    rng = np.random.default_rng(0)
    k = 1.0 / np.sqrt(H)
    ins = {"x": rng.standard_normal((B, T, IN), dtype=np.float32)}
    for l in range(NLAYERS):
        ind = IN if l == 0 else H
        ins[f"w_ih_{l}"] = rng.uniform(-k, k, (G, ind)).astype(np.float32)
        ins[f"w_hh_{l}"] = rng.uniform(-k, k, (G, H)).astype(np.float32)
        ins[f"b_ih_{l}"] = rng.uniform(-k, k, (G,)).astype(np.float32)
        ins[f"b_hh_{l}"] = rng.uniform(-k, k, (G,)).astype(np.float32)
    ins["fc_w"] = rng.uniform(-k, k, (PRED, H)).astype(np.float32)
    ins["fc_b"] = rng.uniform(-k, k, (PRED,)).astype(np.float32)
    print(kernel(**ins).shape)


# revision 6
# speedup vs baseline: 1.0320x; 1.0320x over previous
"""Trainium2 Bass kernel for a 3-layer GRU (B=128, T=512, IN=128, H=1024, PRED=96).

Strategy: data-parallel over batch across 8 NeuronCores (B_core=16).
Per core, per layer:
  phase A (inproj): xi = W_ih @ h_prev_seq as batched N=512 GEMMs (bf16),
      biases (b_ih, + b_hh for r/z gates) folded into the PSUM->SBUF
      evacuation on the scalar engine; xi round-trips through DRAM.
  phase B (recurrence): 512 sequential steps; per step 192 self-loading
      bf16 matmuls (K=128, M=128, N=16) accumulate W_hh @ h_t into one
      PSUM bank laid out gate-major [128, 24 chunks x 16 batch]; gate
      math on DVE/ACT in the same layout; hidden state kept fp32 with a
      bf16 shadow for the tensor engine.
Final FC done on-chip; host only transposes [96,16] -> [16,96] per core.
"""

import numpy as np
from contextlib import ExitStack

import concourse.bass as bass
import concourse.bacc as bacc
import concourse.mybir as mybir
import concourse.tile as tile
from concourse.bass_utils import run_bass_kernel_spmd

try:
    from ml_dtypes import bfloat16 as np_bf16
except ImportError:  # pragma: no cover
    import jax.numpy as jnp

    np_bf16 = jnp.bfloat16

B, T, IN, H, NLAYERS, PRED = 128, 512, 128, 1024, 3, 96
NCORES = 8
BC = B // NCORES  # 16
G = 3 * H  # 3072
NK = H // 128  # 8
NM = G // 128  # 24
BLK = 16  # recurrence steps per For_i iteration
BLKA = 4  # early sub-block of xi steps
F32, BF16 = mybir.dt.float32, mybir.dt.bfloat16
AF = mybir.ActivationFunctionType


def build(T_=T):
    nt = T_ * BC
    nblk = T_ // BLK
    ntile = nt // 512 if nt >= 512 else 0
    nc = bacc.Bacc("TRN2", target_bir_lowering=False, debug=False,
                   num_devices=NCORES)

    xT = nc.dram_tensor("xT", [128, nt], BF16, kind="ExternalInput")
    wihs = [
        nc.dram_tensor("wih0", [128, 1, G], BF16, kind="ExternalInput"),
        nc.dram_tensor("wih1", [128, NK, G], BF16, kind="ExternalInput"),
        nc.dram_tensor("wih2", [128, NK, G], BF16, kind="ExternalInput"),
    ]
    whhs = [nc.dram_tensor(f"whh{l}", [128, NK, G], BF16, kind="ExternalInput")
            for l in range(NLAYERS)]
    bizs = [nc.dram_tensor(f"biz{l}", [128, NM], F32, kind="ExternalInput")
            for l in range(NLAYERS)]
    bhns = [nc.dram_tensor(f"bhn{l}", [128, NK * BC], F32, kind="ExternalInput")
            for l in range(NLAYERS)]
    fcw = nc.dram_tensor("fcw", [128, NK, PRED], BF16, kind="ExternalInput")
    fcb = nc.dram_tensor("fcb", [PRED, 1], F32, kind="ExternalInput")
    out = nc.dram_tensor("out", [PRED, BC], F32, kind="ExternalOutput")

    with tile.TileContext(nc) as tc, ExitStack() as ctx:
        wpool = ctx.enter_context(tc.tile_pool(name="w", bufs=1))
        cpool = ctx.enter_context(tc.tile_pool(name="const", bufs=1))
        xpool = ctx.enter_context(tc.tile_pool(name="xT", bufs=1))
        rpool = ctx.enter_context(tc.tile_pool(name="rhs", bufs=2))
        epool = ctx.enter_context(tc.tile_pool(name="ev", bufs=3))
        xipool = ctx.enter_context(tc.tile_pool(name="xib", bufs=2))
        wnpool = ctx.enter_context(tc.tile_pool(name="win", bufs=2))
        tpool = ctx.enter_context(tc.tile_pool(name="tmp", bufs=2))
        spool = ctx.enter_context(tc.tile_pool(name="state", bufs=1))
        pspool = ctx.enter_context(tc.tile_pool(name="ps", bufs=3, space="PSUM"))
        fcpool = ctx.enter_context(tc.tile_pool(name="psfc", bufs=1, space="PSUM"))
        pgpool = ctx.enter_context(tc.tile_pool(name="pg", bufs=2, space="PSUM"))
        dpool = ctx.enter_context(tc.tile_pool(name="dram", bufs=1, space="DRAM"))
        hqpool = ctx.enter_context(tc.tile_pool(name="hseq", bufs=2, space="DRAM"))

        # persistent state
        h32 = spool.tile([128, NK, BC], F32, tag="h32")
        h16 = spool.tile([128, NK, BC], BF16, tag="h16")

        xT_sb = xpool.tile([128, nt], BF16, tag="xT")
        nc.sync.dma_start(xT_sb[:], xT[:])
        fcw_sb = cpool.tile([128, NK, PRED], BF16, tag="fcw")
        nc.sync.dma_start(fcw_sb[:], fcw[:])
        fcb_sb = cpool.tile([PRED, 1], F32, tag="fcb")
        nc.sync.dma_start(fcb_sb[:], fcb[:])

        hseq_prev = None
        for l in range(NLAYERS):
            nkl = 1 if l == 0 else NK
            wih_sb = wpool.tile([128, NK, G], BF16, tag="wih")
            nc.sync.dma_start(wih_sb[:, 0:nkl, :], wihs[l][:])
            whh_sb = wpool.tile([128, NK, G], BF16, tag="whh")
            nc.sync.dma_start(whh_sb[:], whhs[l][:])
            biz_sb = cpool.tile([128, NM], F32, tag="biz")
            nc.sync.dma_start(biz_sb[:], bizs[l][:])
            bhn_sb = cpool.tile([128, NK * BC], F32, tag="bhn")
            nc.sync.dma_start(bhn_sb[:], bhns[l][:])

            xi = dpool.tile([128, NM, nt], BF16, tag="xi")

            # ---------------- phase A: input projection ----------------
            def inproj_body(n):
                noff = n * 512
                if l == 0:
                    rhs_of = [xT_sb[:, bass.ds(noff, 512)]]
                else:
                    rhsb = rpool.tile([128, NK, 512], BF16, tag="rhs")
                    nc.sync.dma_start(rhsb[:], hseq_prev[:, :, bass.ds(noff, 512)])
                    rhs_of = [rhsb[:, k, :] for k in range(NK)]
                for m in range(NM):
                    ps = pspool.tile([128, 512], F32, tag="ps")
                    for k in range(nkl):
                        nc.tensor.matmul(ps[:], wih_sb[:, k, m * 128:(m + 1) * 128],
                                         rhs_of[k], start=(k == 0), stop=(k == nkl - 1))
                    ev = epool.tile([128, 512], BF16, tag="ev")
                    nc.scalar.activation(ev[:], ps[:], AF.Identity, bias=biz_sb[:, m:m + 1])
                    nc.sync.dma_start(xi[:, m, bass.ds(noff, 512)], ev[:])

            if ntile:
                with tc.For_i(0, ntile) as n:
                    inproj_body(n)
            else:  # tiny-T debug path
                for m in range(NM):
                    ps = pspool.tile([128, nt], F32, tag="ps")
                    if l == 0:
                        for k in range(1):
                            nc.tensor.matmul(ps[:], wih_sb[:, k, m * 128:(m + 1) * 128],
                                             xT_sb[:], start=True, stop=True)
                    else:
                        rhsb = rpool.tile([128, NK, nt], BF16, tag="rhs")
                        nc.sync.dma_start(rhsb[:], hseq_prev[:])
                        for k in range(NK):
                            nc.tensor.matmul(ps[:], wih_sb[:, k, m * 128:(m + 1) * 128],
                                             rhsb[:, k, :], start=(k == 0), stop=(k == NK - 1))
                    ev = epool.tile([128, nt], BF16, tag="ev")
                    nc.scalar.activation(ev[:], ps[:], AF.Identity, bias=biz_sb[:, m:m + 1])
                    nc.sync.dma_start(xi[:, m, :], ev[:])

            # ---------------- phase B: recurrence ----------------
            nc.vector.memset(h32[:], 0.0)
            nc.vector.memset(h16[:], 0.0)
            last = l == NLAYERS - 1
            if not last:
                hseq = hqpool.tile([128, NK, nt], BF16, tag="hseq")

            with tc.For_i(0, nblk) as blk:
                coff = blk * (BLK * BC)
                xiba = xipool.tile([128, NM, BLKA * BC], BF16, tag="xiba")
                nc.sync.dma_start(xiba[:], xi[:, :, bass.ds(coff, BLKA * BC)])
                xibb = xipool.tile([128, NM, (BLK - BLKA) * BC], BF16, tag="xibb")
                nc.sync.dma_start(xibb[:], xi[:, :, bass.ds(coff + BLKA * BC,
                                                            (BLK - BLKA) * BC)])
                if not last:
                    winb = wnpool.tile([128, NK, BLK * BC], BF16, tag="win")
                for dt in range(BLK):
                    pg = pgpool.tile([128, NM, BC], F32, tag="pg")
                    for m in range(NM):
                        for k in range(NK):
                            nc.tensor.matmul(pg[:, m, :],
                                             whh_sb[:, k, m * 128:(m + 1) * 128],
                                             h16[:, k, :],
                                             start=(k == 0), stop=(k == NK - 1))
                    if dt < BLKA:
                        xs = xiba[:, :, dt * BC:(dt + 1) * BC]
                    else:
                        xs = xibb[:, :, (dt - BLKA) * BC:(dt - BLKA + 1) * BC]
                    rzp = tpool.tile([128, 256], F32, tag="rzp")
                    nc.vector.tensor_add(rzp[:], pg[:, 0:16, :], xs[:, 0:16, :])
                    rz = tpool.tile([128, 256], F32, tag="rz")
                    nc.scalar.activation(rz[:], rzp[:], AF.Sigmoid)
                    hnb = tpool.tile([128, 128], F32, tag="hnb")
                    nc.vector.tensor_add(hnb[:], pg[:, 16:24, :], bhn_sb[:])
                    t1 = tpool.tile([128, 128], F32, tag="t1")
                    nc.vector.tensor_mul(t1[:], rz[:, 0:128], hnb[:])
                    t2 = tpool.tile([128, 128], F32, tag="t2")
                    nc.vector.tensor_add(t2[:], t1[:], xs[:, 16:24, :])
                    nt_ = tpool.tile([128, 128], F32, tag="nt")
                    nc.scalar.activation(nt_[:], t2[:], AF.Tanh)
                    d = tpool.tile([128, 128], F32, tag="d")
                    nc.vector.tensor_sub(d[:], h32[:], nt_[:])
                    t4 = tpool.tile([128, 128], F32, tag="t4")
                    nc.vector.tensor_mul(t4[:], d[:], rz[:, 128:256])
                    nc.vector.tensor_add(h16[:], t4[:], nt_[:])
                    nc.vector.tensor_add(h32[:], t4[:], nt_[:])
                    if not last:
                        nc.vector.tensor_copy(winb[:, :, dt * BC:(dt + 1) * BC], h16[:])
                if not last:
                    nc.sync.dma_start(hseq[:, :, bass.ds(coff, BLK * BC)], winb[:])
            if not last:
                hseq_prev = hseq

        # ---------------- final FC ----------------
        psfc = fcpool.tile([PRED, BC], F32, tag="psfc")
        for k in range(NK):
            nc.tensor.matmul(psfc[:], fcw_sb[:, k, :], h16[:, k, :],
                             start=(k == 0), stop=(k == NK - 1))
        ofc = epool.tile([PRED, BC], F32, tag="ofc")
        nc.scalar.activation(ofc[:], psfc[:], AF.Identity, bias=fcb_sb[:])
        nc.sync.dma_start(out[:], ofc[:])

    nc.compile()
    return nc


def prep_inputs(inputs, T_=T):
    """Host-side layout preprocessing. Returns (in_maps, shared)."""
    x = np.asarray(inputs["x"], np.float32)

    def chunkT(w):  # [G_out, K*128] -> [128, K, G_out] (lhsT layout)
        w = np.asarray(w, np.float32)
        gout, kin = w.shape
        return np.ascontiguousarray(
            w.T.reshape(kin // 128, 128, gout).transpose(1, 0, 2)
        ).astype(np_bf16)

    shared = {}
    for l in range(NLAYERS):
        wih = np.asarray(inputs[f"w_ih_{l}"], np.float32)
        shared[f"wih{l}"] = chunkT(wih)
        shared[f"whh{l}"] = chunkT(inputs[f"w_hh_{l}"])
        b_ih = np.asarray(inputs[f"b_ih_{l}"], np.float32)
        b_hh = np.asarray(inputs[f"b_hh_{l}"], np.float32)
        comb = b_ih.copy()
        comb[:2 * H] += b_hh[:2 * H]
        shared[f"biz{l}"] = np.ascontiguousarray(comb.reshape(NM, 128).T)
        bhn = b_hh[2 * H:].reshape(NK, 128).T  # [128, NK]
        shared[f"bhn{l}"] = np.ascontiguousarray(np.repeat(bhn, BC, axis=1))
    shared["fcw"] = chunkT(inputs["fc_w"])
    shared["fcb"] = np.asarray(inputs["fc_b"], np.float32).reshape(PRED, 1)

    in_maps = []
    for c in range(NCORES):
        xc = x[c * BC:(c + 1) * BC, :T_, :]  # [BC, T, IN]
        xTc = np.ascontiguousarray(
            xc.transpose(2, 1, 0).reshape(IN, T_ * BC)
        ).astype(np_bf16)
        m = dict(shared)
        m["xT"] = xTc
        in_maps.append(m)
    return in_maps


_NC_CACHE = {}


def kernel(**inputs):
    if "nc" not in _NC_CACHE:
        _NC_CACHE["nc"] = build()
    nc = _NC_CACHE["nc"]
    in_maps = prep_inputs(inputs)
    res = run_bass_kernel_spmd(nc, in_maps, list(range(NCORES)))
    outs = []
    for c in range(NCORES):
        o = np.asarray(res.results[c]["out"], np.float32)  # [PRED, BC]
        outs.append(o.T)  # [BC, PRED]
    return np.concatenate(outs, axis=0)  # [B, PRED]


if __name__ == "__main__":
    rng = np.random.default_rng(0)
    k = 1.0 / np.sqrt(H)
    ins = {"x": rng.standard_normal((B, T, IN), dtype=np.float32)}
    for l in range(NLAYERS):
        ind = IN if l == 0 else H
        ins[f"w_ih_{l}"] = rng.uniform(-k, k, (G, ind)).astype(np.float32)
        ins[f"w_hh_{l}"] = rng.uniform(-k, k, (G, H)).astype(np.float32)
        ins[f"b_ih_{l}"] = rng.uniform(-k, k, (G,)).astype(np.float32)
        ins[f"b_hh_{l}"] = rng.uniform(-k, k, (G,)).astype(np.float32)
    ins["fc_w"] = rng.uniform(-k, k, (PRED, H)).astype(np.float32)
    ins["fc_b"] = rng.uniform(-k, k, (PRED,)).astype(np.float32)
    print(kernel(**ins).shape)
